# revision 39
# baseline (speedup 1.0000x reference)
"""Causal self-attention on 8 TRN2 NeuronCores, batch-data-parallel (one batch
element per core).

Layout strategy (per core, S=1024, D=1024, H=16, hd=64):
  - Host pre-transposes x -> xT [D,S] and all weights -> [in_dim, out_dim].
  - qk projection produces q,k transposed ([e,s]) per head-pair: lhsT = wqkT
    tiles, rhs = xT.  Head h lives at partitions 64*(h%2)..+64.
  - v natural [s,e]: lhsT = xT tiles, rhs = resident wv half tiles; stored
    interleaved with a ones column per head (65 cols/head) so the AV matmul's
    PSUM row 64 is the softmax denominator (rowsum of unnormalized attn).
  - scoresT [sk,sq] per head-pair via K=64 matmuls; exp on ACT (scale=1/8
    folded in); causal diag masked by multiplicative upper-tri mask (DVE);
    fully-masked tiles never computed.
  - AV: outT'[hd+1, sq] accumulated m-major in 512-wide chunks; normalization
    via approx-reciprocal of the den row + PE rank-1 broadcast + DVE multiply.
  - proj: y[s,e] with lhsT = outT tiles, rhs = wpT (DMA'd over the dead xT
    SBUF space during pairs 6-7) + rank-1 bias term (beff = b_proj +
    W_proj @ b_v; b_v folds exactly through softmax rowsum).
  - QKV matmul quanta are interleaved into the attention pair loop so the PE
    stream stays dense (HAM clock gate stays 8/8). Weight DMAs are hoisted
    and issued 1+ pairs ahead of their consuming matmuls.
  - PSUM-eviction copies / bias adds run on the otherwise-idle Pool engine
    (nc.gpsimd) or ACT, keeping DVE for mask/normalize only.
All matmuls run in float32r (TF32-like, full PE rate at N>=256).
"""

import numpy as np

B, S, D, H = 8, 1024, 1024, 16
HD = D // H          # 64
P = 128
NCORES = 8
KO = D // P          # 8 contraction tiles over d
MT = (2 * D) // P    # 16 m-tiles for q,k
ST = S // P          # 8 s-tiles
NPAIRS = H // 2      # 8 head pairs

_CACHE = {}
TRACE = False        # set by test harness to collect an NTFF profile


def _score_chunks(w):
    # split w into pieces, each >=256 when possible (fp32r full-rate needs
    # moving dim >=256), <=512 (PSUM bank limit)
    table = {1024: [512, 512], 896: [512, 384], 768: [512, 256],
             640: [384, 256], 512: [512], 384: [384], 256: [256], 128: [128]}
    return table[w]


def _build():
    import concourse.tile as tile
    from concourse import bacc, mybir

    F32R = mybir.dt.float32r
    F32 = mybir.dt.float32
    AF = mybir.ActivationFunctionType

    nc = bacc.Bacc("TRN2", target_bir_lowering=False, debug=False,
                   num_devices=NCORES)
    xT_d = nc.dram_tensor("xT", [D, S], F32R, kind="ExternalInput").ap()
    # wqkT host-laid-out as [p, m, ko, e] so each m-tile's DMA reads
    # contiguous 4KB per partition (512B lines throttle the DMA engines)
    wqkT_d = nc.dram_tensor("wqkT", [P, MT, KO, P], F32R,
                            kind="ExternalInput").ap()
    wvT_d = nc.dram_tensor("wvT", [D, D], F32R, kind="ExternalInput").ap()
    wpT_d = nc.dram_tensor("wpT", [D, D], F32R, kind="ExternalInput").ap()
    bqk_d = nc.dram_tensor("bqk", [2 * D], F32, kind="ExternalInput").ap()
    beff_d = nc.dram_tensor("beff", [D], F32R, kind="ExternalInput").ap()
    umask_d = nc.dram_tensor("umask", [P, P], F32, kind="ExternalInput").ap()
    y_d = nc.dram_tensor("y", [S, D], F32, kind="ExternalOutput").ap()

    wvT_v = wvT_d.rearrange("(ko p) e -> p ko e", p=P)
    wpT_v = wpT_d.rearrange("(ko p) e -> p ko e", p=P)
    xT_v = xT_d.rearrange("(ko p) s -> p ko s", p=P)

    with tile.TileContext(nc) as tc:
        with (
            tc.tile_pool(name="bigio", bufs=1) as bigio,
            tc.tile_pool(name="qkp", bufs=3) as qkp,
            tc.tile_pool(name="vp", bufs=1) as vpool,
            tc.tile_pool(name="wqk", bufs=4) as wqkp,
            tc.tile_pool(name="wvp", bufs=1) as wvp,
            tc.tile_pool(name="attn", bufs=5) as attnp,
            tc.tile_pool(name="rt", bufs=2) as rtp,
            tc.tile_pool(name="rb", bufs=1) as rbp,
            tc.tile_pool(name="todd", bufs=1) as toddp,
            tc.tile_pool(name="ystg", bufs=2) as ystgp,
            tc.tile_pool(name="avsb", bufs=2) as avsbp,
            tc.tile_pool(name="cst", bufs=1) as cst,
            tc.tile_pool(name="psS", bufs=4, space="PSUM") as psS,
            tc.tile_pool(name="psAV", bufs=2, space="PSUM") as psAV,
            tc.tile_pool(name="ydram", bufs=1, space="DRAM") as ydramp,
        ):
            # ---------- constants ----------
            umask = cst.tile([P, P], F32)
            nc.sync.dma_start(umask[:], umask_d)
            bqk_sb = cst.tile([P, MT], F32)
            nc.sync.dma_start(bqk_sb[:], bqk_d.rearrange("(m p) -> p m", p=P))
            beff_sb = cst.tile([1, D], F32R)
            nc.sync.dma_start(beff_sb[:], beff_d[None, :])
            onecol = cst.tile([P, 1], F32)
            nc.vector.memset(onecol[:], 1.0)
            ones1x128 = cst.tile([1, P], F32R)
            nc.vector.tensor_copy(
                ones1x128[:], onecol[0:1, :].broadcast_to([1, P]))
            ones65r = cst.tile([65, 64], F32R)
            nc.vector.memset(ones65r[64:65, :].bitcast(F32), 1.0)
            zcol = cst.tile([P, 1], F32R)
            nc.vector.memset(zcol[:].bitcast(F32), 0.0)

            # ---------- big SBUF residents ----------
            # xT is overwritten with wpT during pairs 6-7 (QKV matmuls are
            # done reading it by then); proj reads it as the wpT resident.
            xT = bigio.tile([P, KO, S], F32R, tag="xT")
            outT = bigio.tile([P, KO, S], F32R, tag="outT")
            v_sb = vpool.tile([P, ST, H * (HD + 1)], F32R)
            v_hview = v_sb[:].rearrange("p st (h c) -> p st h c", c=HD + 1)

            # ---------- input DMA schedule ----------
            # first qk quantum needs xT ko=0 + wqk m=0 first; stream the rest
            wqk_tiles = {}

            def wqk_dma(j):
                for part in (0, 1):
                    m = j if part == 0 else NPAIRS + j
                    wt = wqkp.tile([P, KO, P], F32R, tag="wqk",
                                   name=f"wqk{m}")
                    nc.sync.dma_start(wt[:], wqkT_d[:, m, :, :])
                    wqk_tiles[m] = wt

            wv_res = {}

            def wv_dma(nE):
                wt = wvp.tile([P, KO, 512], F32R, tag="wv", name=f"wv{nE}")
                for kog in range(KO // 2):
                    nc.sync.dma_start(
                        wt[:, 2 * kog:2 * kog + 2, :],
                        wvT_v[:, 2 * kog:2 * kog + 2,
                              nE * 512:(nE + 1) * 512])
                wv_res[nE] = wt

            wqk_dma(0)
            for ko in range(KO):
                for h in range(2):
                    nc.sync.dma_start(xT[:, ko, h * 512:(h + 1) * 512],
                                      xT_v[:, ko, h * 512:(h + 1) * 512])
                if ko == 0:
                    wqk_dma(1)
            wv_dma(0)
            nc.vector.tensor_copy(
                v_hview[:, :, :, HD:HD + 1],
                onecol[:, None, None, :].broadcast_to([P, ST, H, 1]))

            qk_tiles = {}    # j -> [128, 2, S] tile (0=q, 1=k)

            # ---------- QKV work quanta (emitted interleaved) ----------
            def qk_quanta(j):
                # 4 closures; each computes one (part, nn) psum group.
                # Weights were DMA'd earlier via wqk_dma(j).
                t = qkp.tile([P, 2, S], F32R, tag="qkt", name=f"qk{j}")
                qk_tiles[j] = t

                def quantum(part, nn):    # part 0=q (m-tile j), 1=k (8+j)
                    def go():
                        m = j if part == 0 else NPAIRS + j
                        wt = wqk_tiles[m]
                        ps = psS.tile([P, 512], F32, tag="ps", name=f"qkps{m}")
                        for ko in range(KO):
                            nc.tensor.matmul(
                                ps[:], wt[:, ko, :],
                                xT[:, ko, nn * 512:(nn + 1) * 512],
                                start=(ko == 0), stop=(ko == KO - 1))
                        nc.vector.tensor_scalar_add(
                            t[:, part, nn * 512:(nn + 1) * 512], ps[:],
                            bqk_sb[:, m:m + 1])
                    return go
                return [quantum(0, 0), quantum(0, 1),
                        quantum(1, 0), quantum(1, 1)]

            def v_quanta(nE):
                # v half nE: e_v cols 512*nE.. (heads 8nE..8nE+7), 8 quanta
                # of 1 s-tile each (holds only one psS buf at a time),
                # reading the resident wv half tile
                def quantum(st):
                    def go():
                        wt = wv_res[nE]
                        ps = psS.tile([P, 512], F32, tag="ps",
                                      name=f"vps{nE}_{st}")
                        for ko in range(KO):
                            nc.tensor.matmul(
                                ps[:], xT[:, ko, st * P:(st + 1) * P],
                                wt[:, ko, :], start=(ko == 0),
                                stop=(ko == KO - 1))
                        nc.vector.tensor_copy(
                            v_hview[:, st, 8 * nE:8 * (nE + 1), 0:HD],
                            ps[:].rearrange("p (h c) -> p h c", c=HD))
                    return go
                return [quantum(st) for st in range(ST)]

            # ---------- attention ----------
            pend = {}

            def scores_exp(j, m):
                qk_t = qk_tiles[j]
                w = S - m * P
                for hb, base in ((0, 0), (1, 64)):   # head 2j+hb
                    at = attnp.tile([P, S], F32R, tag="at",
                                    name=f"at{j}_{hb}_{m}")
                    pend[(j, hb, m)] = at
                    gw = m * P - (0 if m <= 3 else 512)
                    if 0 < gw < 512:
                        nc.gpsimd.tensor_copy(
                            at[:, m * P - gw:m * P],
                            zcol[:].broadcast_to([P, gw]))
                    off = m * P
                    for cw in _score_chunks(w):
                        ps = psS.tile([P, 512], F32, tag="ps",
                                      name=f"sps{j}_{hb}_{m}")
                        nc.tensor.matmul(
                            ps[:, 0:cw],
                            qk_t[base:base + 64, 1, m * P:(m + 1) * P],
                            qk_t[base:base + 64, 0, off:off + cw],
                            start=True, stop=True)
                        nc.scalar.activation(
                            at[:, off:off + cw], ps[:, 0:cw], AF.Exp,
                            scale=0.125)
                        off += cw
                    nc.vector.tensor_mul(
                        at[:, m * P:(m + 1) * P], at[:, m * P:(m + 1) * P],
                        umask[:])

            def av_m(j, m):
                st8 = pend[f"ps{j}"]
                for hb in (0, 1):
                    h = 2 * j + hb
                    at = pend[(j, hb, m)]
                    for n in range((0 if m <= 3 else 1), 2):
                        nc.tensor.matmul(
                            st8[hb][:, n * 512:(n + 1) * 512],
                            v_sb[:, m, h * (HD + 1):(h + 1) * (HD + 1)],
                            at[:, n * 512:(n + 1) * 512],
                            start=(m == 0), stop=(m == 4 * n + 3))

            def evict_recip(j):
                # move the [65, S] AV accumulators out of PSUM (on Pool) so
                # the next pair's AV matmuls get the PSUM slots immediately,
                # then take the reciprocal of the den row (approx_fast:
                # ~4e-6 rel; den >= exp(0) > 0 so no edge cases)
                avcs, recs = [], []
                for hb in (0, 1):
                    avc = avsbp.tile([65, S], F32R, tag="avc",
                                     name=f"avc{j}_{hb}")
                    # split each eviction's halves across ACT and DVE so
                    # they run in parallel (they gate the next pair's AV)
                    src = pend[f"ps{j}"][hb]
                    eng = (nc.scalar.copy, nc.vector.tensor_copy)
                    for c in range(2):
                        eng[(hb + c) % 2](
                            avc[:, c * 512:(c + 1) * 512],
                            src[:, c * 512:(c + 1) * 512])
                    avcs.append(avc)
                    rt = rtp.tile([65, S], F32R, tag="rt")
                    # custom-DVE op misbehaves on single-partition APs on HW:
                    # run it over all 65 rows (lanes are parallel) and consume
                    # only the den row (64); other lanes are never read
                    for c in range(4):
                        rt32 = rtp.tile([65, 256], F32, tag="rt32", bufs=1)
                        nc.vector.reciprocal_approx_fast(
                            rt32[:],
                            avc[:, c * 256:(c + 1) * 256].bitcast(F32))
                        nc.vector.tensor_copy(
                            rt[64:65, c * 256:(c + 1) * 256], rt32[64:65, :])
                    recs.append(rt)
                pend[f"avc{j}"] = avcs
                pend[f"rec{j}"] = recs
                del pend[f"ps{j}"]

            def rb_norm(j):
                for hb in (0, 1):
                    rt = pend[f"rec{j}"][hb]
                    rb_t = rbp.tile([64, S], F32R, tag="rb")
                    for c in range(2):
                        rps = psS.tile([P, 512], F32, tag="ps",
                                       name=f"rbps{j}_{hb}_{c}")
                        nc.tensor.matmul(
                            rps[0:64, :], ones65r[64:65, :],
                            rt[64:65, c * 512:(c + 1) * 512],
                            start=True, stop=True)
                        nc.scalar.copy(
                            rb_t[:, c * 512:(c + 1) * 512], rps[0:64, :])
                    avc = pend[f"avc{j}"][hb]
                    if hb == 0:
                        nc.gpsimd.tensor_mul(
                            outT[0:64, j, :], avc[0:64, :], rb_t[:])
                    else:
                        # lanes cannot shift partitions: multiply to an
                        # SBUF tmp, then DMA-shift rows 0..63 -> 64..127
                        tmp = toddp.tile([64, S], F32R, tag="todd")
                        nc.gpsimd.tensor_mul(tmp[:], avc[0:64, :], rb_t[:])
                        nc.sync.dma_start(outT[64:128, j, :], tmp[:])
                del pend[f"avc{j}"], pend[f"rec{j}"]

            # ---------- projection partials (tail PE filler) ----------
            # During pairs 5-7 the qk/v quanta are exhausted; to keep the
            # PE stream dense (HAM clock at 8/8), emit partial projection
            # sums over the finished outT slices (ko 0..4 + bias) into a
            # DRAM scratch, finished after the loop with ko 5..7.
            ypart = ydramp.tile([S, D], F32, tag="ypart")
            KPART = 5

            def proj_partial(st, nE):
                def go():
                    # nE=0 weights come from the wv resident (re-loaded with
                    # wpT half 0 after the v quanta retire); nE=1 from the
                    # xT alias (re-loaded with full wpT during pair 6)
                    ps = psS.tile([P, 512], F32, tag="ps",
                                  name=f"pp{st}_{nE}")
                    for ko in range(KPART):
                        w = (wv_res["wp0"][:, ko, :] if nE == 0
                             else xT[:, ko, 512:1024])
                        nc.tensor.matmul(
                            ps[:], outT[:, ko, st * P:(st + 1) * P], w,
                            start=(ko == 0), stop=False)
                    nc.tensor.matmul(
                        ps[:], ones1x128[:],
                        beff_sb[:, nE * 512:(nE + 1) * 512],
                        start=False, stop=True)
                    ystg = ystgp.tile([P, 512], F32, tag="ystg",
                                      name=f"pstg{st}_{nE}")
                    nc.vector.tensor_copy(ystg[:], ps[:])
                    nc.sync.dma_start(
                        ypart[st * P:(st + 1) * P,
                              nE * 512:(nE + 1) * 512], ystg[:])
                return go

            def wp0_dma():
                wt = wvp.tile([P, KO, 512], F32R, tag="wv", name="wp0")
                nc.sync.dma_start(wt[:], wpT_v[:, :, 0:512])
                wv_res["wp0"] = wt

            # ---------- interleaved emission ----------
            # prologue: qk for pairs 0,1 and v half 0; weight DMAs for
            # pairs 2,3 interleaved so they land ~2 pairs ahead of use
            for q in qk_quanta(0):
                q()
            wqk_dma(2)
            for q in qk_quanta(1):
                q()
            for q in v_quanta(0):
                q()
            wv_dma(1)
            vwork = list(v_quanta(1))   # needed from pair 4 on

            for j in range(NPAIRS):
                # qkv work to interleave into this pair's m-steps; the
                # weight DMA for pair j+3 goes first so it lands a full
                # pair ahead of its consuming quanta (emitted at pair j+1)
                work = []
                if j + 3 < NPAIRS:
                    work.append(lambda j=j: wqk_dma(j + 3))
                if j + 2 < NPAIRS:
                    work.extend(qk_quanta(j + 2))
                if j in (1, 2) and vwork:
                    for _ in range(4):
                        work.append(vwork.pop(0))
                if j == 3:
                    work.append(wp0_dma)
                if j == 5:
                    # rb_norm(4) lands at m=4, so these pop at m>=5
                    work.extend([proj_partial(0, 0), proj_partial(1, 0)])
                if j == 6:
                    # xT is dead (all QKV matmuls emitted); stream wpT into
                    # its SBUF space, interleaved with nE=0 partials
                    for ko in range(KO):
                        work.append(lambda ko=ko: nc.sync.dma_start(
                            xT[:, ko, :], wpT_v[:, ko, :]))
                        if 2 + ko <= 7:
                            work.append(proj_partial(2 + ko, 0))
                if j == 7:
                    work.extend([proj_partial(st, 1) for st in range(ST)])
                for m in range(ST):
                    scores_exp(j, m)
                    if m == 4 and j > 0:
                        rb_norm(j - 1)
                    if m == 0:
                        pend[f"ps{j}"] = [
                            psAV.tile([65, S], F32, tag="av",
                                      name=f"av{j}_{hb}") for hb in range(2)]
                    if m >= 2:
                        av_m(j, m - 2)
                    if m % 2 == 1 and work:
                        # drain evenly over the remaining odd slots so no
                        # burst of quanta piles up at the pair boundary
                        slots_left = (ST - m + 1) // 2
                        npop = -(-len(work) // slots_left)
                        for _ in range(npop):
                            if work:
                                work.pop(0)()
                av_m(j, ST - 2)
                while work:
                    work.pop(0)()
                av_m(j, ST - 1)
                evict_recip(j)
            rb_norm(NPAIRS - 1)

            # ---------- output projection: finish ko 5..7 + partials ----------
            for st in range(ST):
                for nE in range(2):
                    ps = psS.tile([P, 512], F32, tag="ps",
                                  name=f"yps{st}_{nE}")
                    for ko in range(KPART, KO):
                        nc.tensor.matmul(
                            ps[:], outT[:, ko, st * P:(st + 1) * P],
                            xT[:, ko, nE * 512:(nE + 1) * 512],
                            start=(ko == KPART), stop=(ko == KO - 1))
                    part = ystgp.tile([P, 512], F32, tag="ystg",
                                      name=f"yfin{st}_{nE}")
                    nc.sync.dma_start(
                        part[:], ypart[st * P:(st + 1) * P,
                                       nE * 512:(nE + 1) * 512])
                    nc.vector.tensor_add(part[:], part[:], ps[:])
                    nc.sync.dma_start(
                        y_d[st * P:(st + 1) * P, nE * 512:(nE + 1) * 512],
                        part[:])

    nc.compile()
    return nc


def kernel(x, w_attn, b_attn, w_proj, b_proj):
    import concourse.bass_utils as bass_utils

    if "nc" not in _CACHE:
        _CACHE["nc"] = _build()
    nc = _CACHE["nc"]

    x = np.asarray(x, dtype=np.float32)
    w_attn = np.asarray(w_attn, dtype=np.float32)
    b_attn = np.asarray(b_attn, dtype=np.float32)
    w_proj = np.asarray(w_proj, dtype=np.float32)
    b_proj = np.asarray(b_proj, dtype=np.float32)

    xT = np.ascontiguousarray(np.transpose(x, (0, 2, 1)))        # [B, D, S]
    # [D, 2D] -> [p, m, ko, e] so each m-tile is contiguous per partition
    wqkT = np.ascontiguousarray(
        w_attn[:2 * D].T.reshape(KO, P, MT, P).transpose(1, 2, 0, 3))
    wvT = np.ascontiguousarray(w_attn[2 * D:].T)                 # [D, D]
    wpT = np.ascontiguousarray(w_proj.T)                         # [D, D]
    bqk = np.ascontiguousarray(b_attn[:2 * D])
    bv = b_attn[2 * D:]
    beff = (b_proj.astype(np.float64)
            + w_proj.astype(np.float64) @ bv.astype(np.float64)
            ).astype(np.float32)
    umask = np.triu(np.ones((P, P), dtype=np.float32))           # f >= p
    in_maps = [
        dict(xT=xT[b], wqkT=wqkT, wvT=wvT, wpT=wpT, bqk=bqk, beff=beff,
             umask=umask)
        for b in range(B)
    ]
    res = bass_utils.run_bass_kernel_spmd(
        nc, in_maps, core_ids=list(range(NCORES)), trace=TRACE)
    if TRACE:
        _CACHE["exec_time_ns"] = res.exec_time_ns
        _CACHE["trace"] = res.instructions_and_trace
    return np.stack([res.results[b]["y"] for b in range(B)], axis=0)
